# revision 43
# baseline (speedup 1.0000x reference)
"""Bi-attention kernel for Trainium2 (Bass/Tile), 8-core data-parallel over batch.

Problem (per batch element b, full shapes x:[8,2048,1024] f32, mask:[8,2048] i32):
    score   = x_b @ x_b.T                      [2048, 2048]
    score   = where(mask==0, -inf, score)      (mask keys)
    attn    = softmax(score, axis=-1)
    context = attn @ x_b                       [2048, 1024]
    out_b   = concat([x, ctx, x+ctx, x-ctx, x*ctx], -1)   [2048, 5120]

Sharding: batch dim (8) across the 8 NeuronCores, one batch element per core.
No cross-core communication.

Per-core schedule (S=2048, D=1024, P=128), exploiting score symmetry:
  The raw fp16 score matrix snat[p, t, k] = s[t*128+p, k] is materialized
  once.  Only the upper-triangle tiles (i,j), j>=i are computed by matmul
  (136 of 256); each lower tile (t,i), t>i is a single PE transpose of its
  mirror (score symmetry), which halves the score-matmul PE work.

  Per row-tile i:
    A(i): fp16 score matmuls into f32 PSUM for tiles (i, j>=i), ACT-drained
          into snat; PE transposes of (i, t>i) DVE-drained into snat; an
          early DVE add+reduce over the mirror columns [0, i*128)
          (resident since A(j<i)) banks a partial masked row max into a
          per-row smask buffer (per-row so lookahead writers can't clobber
          a row the exp hasn't consumed).
    B(i): a short DVE add+reduce over the direct columns completes the
          masked row max; ACT computes p = exp(smask - m) in two
          1024-halves with accum_out giving the denominators for free;
          PE transposes p (2 batches of 8 through one fp16 PSUM bank)
          into pT with split half-drains; 2 x 16 fp16 context
          matmuls into [128,512] f32 PSUM, ACT-drained scaled by 1/denom.
          Masked keys are exact zeros in p (additive -30000 before exp),
          so numerator and denominator are consistent; rows whose own key
          is live are exactly one-hot (the diagonal dominates by ~30
          sigma) and reproduce x bit-accurately through the fp16 weights.
    out:  x block via DMA round-trip; x+-* blocks on Pool/DVE; one wide
          DMA per row (fine-grained on the last row for a short flush).

  Emission order pipelines A two rows ahead of B so the B(i) chain
  (reduce -> exp -> p-transpose) hides under A(i+1/2) and B(i-1) PE work.
  Setup interleaves x loads, fp16 casts, xT PE transposes, and the score
  chunks whose key range is already resident.
"""

import os

os.environ.setdefault("JAX_PLATFORMS", "axon")  # NEFF executes via the axon PJRT tunnel

import numpy as np

import concourse.bass as bass
import concourse.tile as tile
from concourse import bacc, mybir
from concourse.bass_utils import run_bass_kernel_spmd
from concourse.masks import make_identity

P = 128
S = 2048
D = 1024
NT = S // P          # 16 token tiles
KD = D // P          # 8 d subtiles (score contraction)
NB = 8               # batch / cores
DT = mybir.dt
MASK_NEG = -30000.0  # fp16-safe additive key mask


def _build():
    nc = bacc.Bacc()
    x = nc.dram_tensor("x", (S, D), DT.float32, kind="ExternalInput")
    mask = nc.dram_tensor("mask", (S,), DT.int32, kind="ExternalInput")
    out = nc.dram_tensor("out", (S, 5 * D), DT.float32, kind="ExternalOutput")

    with tile.TileContext(nc) as tc:
        with (
            tc.tile_pool(name="const", bufs=1) as const,
            tc.tile_pool(name="ps_s", bufs=3, space="PSUM") as ps_s,
            tc.tile_pool(name="ps_t", bufs=3, space="PSUM") as ps_t,
            tc.tile_pool(name="ps_c", bufs=2, space="PSUM") as ps_c,
        ):
            warm = const.tile([P, 1], DT.float32)
            nc.gpsimd.memset(warm[:], 0.0)
            warm2 = const.tile([P, 1], DT.float32)
            nc.scalar.copy(warm2[:], warm[:])   # hoist ACT table load
            identf = const.tile([P, P], DT.float32)
            make_identity(nc, identf)
            ident16 = const.tile([P, P], DT.float16)
            nc.vector.tensor_copy(ident16[:], identf[:])

            xT = const.tile([P, KD, S], DT.float16)     # x^T (d on partitions)
            xnb = const.tile([P, NT, D], DT.float16)    # x natural fp16
            snat = const.tile([P, NT, S], DT.float16)   # raw score matrix
            colmask = const.tile([P, S], DT.float16)    # additive key mask

            with tc.tile_pool(name="setup", bufs=1) as setup, \
                 tc.tile_pool(name="xin_pool", bufs=2) as xin_pool, \
                 tc.tile_pool(name="work", bufs=1) as work, \
                 tc.tile_pool(name="swork", bufs=3) as swork, \
                 tc.tile_pool(name="pwork", bufs=2) as pwork, \
                 tc.tile_pool(name="owork", bufs=2) as owork, \
                 tc.tile_pool(name="xwork", bufs=2) as xwork, \
                 tc.tile_pool(name="stats", bufs=4) as stats:

                pbf = work.tile([P, S], DT.float16, name="pbf")      # softmax numerators

                def emit_masks():
                    mask_ap = mask[:]
                    mask_i8 = setup.tile([P, S], DT.int8, tag="mask_i8")
                    nc.gpsimd.dma_start(   # casting broadcast across partitions
                        out=mask_i8[:],
                        in_=bass.AP(tensor=mask_ap.tensor, offset=mask_ap.offset,
                                    ap=[[0, P], mask_ap.ap[0]]),
                    )
                    nc.vector.tensor_scalar(
                        out=colmask[:], in0=mask_i8[:],
                        scalar1=1.0, scalar2=-MASK_NEG,
                        op0=mybir.AluOpType.subtract, op1=mybir.AluOpType.mult,
                    )

                # --- score helpers ----------------------------------------
                def score_chunk(i, c):
                    """Keys [i*P + 512c, +cw) of row-tile i (PSUM f32, ACT drain)."""
                    base = i * P + c * 512
                    cw = min(512, S - base)
                    pss = ps_s.tile([P, 512], DT.float32, tag="pss",
                                    name=f"pss{i}_{c}")
                    for j in range(KD):
                        nc.tensor.matmul(
                            pss[:, :cw],
                            xT[:, j, i * P:(i + 1) * P],
                            xT[:, j, base:base + cw],
                            start=(j == 0),
                            stop=(j == KD - 1),
                        )
                    nc.scalar.copy(snat[:, i, base:base + cw], pss[:, :cw])

                def n_chunks(i):
                    return (S - i * P + 511) // 512

                def emit_mirrors(i):
                    """T(t,i) = M(i,t)^T into snat[:, t, i-block] for t > i."""
                    ts = list(range(i + 1, NT))
                    for b0 in range(0, len(ts), 8):
                        grp = ts[b0:b0 + 8]
                        pst = ps_t.tile([P, 8 * P], DT.float16, tag="pst",
                                        name=f"mir{i}_{b0}")
                        for g, t in enumerate(grp):
                            nc.tensor.transpose(
                                pst[:, g * P:(g + 1) * P],
                                snat[:, i, t * P:(t + 1) * P],
                                ident16[:],
                            )
                        dst = snat[:, grp[0]:grp[0] + len(grp), i * P:(i + 1) * P]
                        src = pst[:, :len(grp) * P].rearrange(
                            "p (b q) -> p b q", b=len(grp))
                        nc.vector.tensor_copy(dst, src)

                # --- setup: stream x, cast, transpose, early score chunks --
                # score chunk (i, c) needs x chunks <= i + 4c + 3
                early = {}
                for k in range(NT):
                    early[k] = [(i, c) for i in range(NT) for c in range(n_chunks(i))
                                if i + 4 * c + 3 == k]

                for ci in range(NT):
                    xin = xin_pool.tile([P, D], DT.float32, tag="xin")
                    nc.sync.dma_start(xin[:, 0:512], x[ci * P:(ci + 1) * P, 0:512])
                    nc.scalar.copy(xnb[:, ci, 0:512], xin[:, 0:512])
                    nc.sync.dma_start(xin[:, 512:D], x[ci * P:(ci + 1) * P, 512:D])
                    nc.scalar.copy(xnb[:, ci, 512:D], xin[:, 512:D])
                    if ci == 1:
                        emit_masks()
                    for jb in range(2):
                        pst = ps_t.tile([P, 8 * P], DT.float16, tag="pst",
                                        name=f"xt{ci}_{jb}")
                        for j4 in range(4):
                            j = jb * 4 + j4
                            nc.tensor.transpose(
                                pst[:, j4 * P:(j4 + 1) * P],
                                xnb[:, ci, j * P:(j + 1) * P],
                                ident16[:],
                            )
                        dst = xT[:, jb * 4:(jb + 1) * 4, ci * P:(ci + 1) * P]
                        src = pst[:, 0:4 * P].rearrange("p (j q) -> p j q", j=4)
                        nc.vector.tensor_copy(dst, src)
                    for (i, c) in early[ci]:
                        score_chunk(i, c)

                emitted = {(i, c) for k in range(NT) for (i, c) in early[k]}

                m12s = {}
                smasks = {}
                xblks = {}

                def emit_xblk(i):
                    # x block round-trip is compute-independent: emit a row
                    # early so the store never lands in the flush window
                    xblk = xwork.tile([P, D], DT.float32, tag="xblk",
                                      name=f"xblk{i}")
                    xblks[i] = xblk
                    nc.sync.dma_start(xblk[:], x[i * P:(i + 1) * P, :])
                    nc.sync.dma_start(out[i * P:(i + 1) * P, 0:D], xblk[:])

                def emit_scores_rest(i):
                    for c in range(n_chunks(i)):
                        if (i, c) not in emitted:
                            score_chunk(i, c)
                    # early partial -max over the mirror columns [0, i*P)
                    # (written by A(j<i) long ago): B(i)'s post-A reduce is
                    # then only the short direct-column piece.
                    m12 = stats.tile([P, 2], DT.float32, tag="m12", name=f"m12{i}")
                    m12s[i] = m12
                    smask = swork.tile([P, S], DT.float16, tag="smask",
                                       name=f"smask{i}")
                    smasks[i] = smask
                    if i > 0:
                        nc.vector.tensor_add(
                            smask[:, 0:i * P], snat[:, i, 0:i * P],
                            colmask[:, 0:i * P],
                        )
                        nc.vector.tensor_reduce(
                            out=m12[:, 0:1], in_=smask[:, 0:i * P],
                            op=mybir.AluOpType.max, axis=mybir.AxisListType.X,
                        )
                    else:
                        nc.vector.memset(m12[:, 0:1], -60000.0)
                    emit_mirrors(i)

                def emit_rest(i, prev_tail, last=False):
                    q_sl = slice(i * P, (i + 1) * P)
                    # finish -(masked row max); smask holds -(s + colmask)
                    m12 = m12s[i]
                    smask = smasks[i]
                    nc.vector.tensor_add(
                        smask[:, i * P:S], snat[:, i, i * P:S],
                        colmask[:, i * P:S],
                    )
                    nc.vector.tensor_reduce(
                        out=m12[:, 1:2], in_=smask[:, i * P:S],
                        op=mybir.AluOpType.max, axis=mybir.AxisListType.X,
                    )
                    mrow = stats.tile([P, 1], DT.float32, tag="mrow",
                                      name=f"mrow{i}")
                    nc.vector.tensor_reduce(
                        out=mrow[:], in_=m12[:],
                        op=mybir.AluOpType.max, axis=mybir.AxisListType.X,
                    )
                    negm = stats.tile([P, 1], DT.float32, tag="negm",
                                      name=f"negm{i}")
                    nc.vector.tensor_scalar_mul(negm[:], mrow[:], -1.0)

                    # p = exp(s + colmask - m) per 1024-half; denominators via
                    # accum_out (masked keys are exact zeros)
                    dsum = stats.tile([P, 2], DT.float32, tag="dsum",
                                      name=f"dsum{i}")
                    for h in range(2):
                        hsl = slice(h * 1024, (h + 1) * 1024)
                        nc.scalar.activation(
                            out=pbf[:, hsl], in_=smask[:, hsl],
                            func=mybir.ActivationFunctionType.Exp,
                            bias=negm[:], scale=1.0,
                            accum_out=dsum[:, h:h + 1],
                        )
                    den = stats.tile([P, 1], DT.float32, tag="den", name=f"den{i}")
                    nc.vector.tensor_reduce(
                        out=den[:], in_=dsum[:],
                        op=mybir.AluOpType.add, axis=mybir.AxisListType.X,
                    )
                    recip = stats.tile([P, 1], DT.float32, tag="recip",
                                       name=f"recip{i}")
                    nc.vector.reciprocal(recip[:], den[:])

                    # transpose p into pT (2 batches of 8 tiles, fp16 PSUM)
                    pT = pwork.tile([P, NT, P], DT.float16, tag="pT", name=f"pT{i}")
                    for b in range(2):
                        pst = ps_t.tile([P, 8 * P], DT.float16, tag="pst",
                                        name=f"pstp{i}_{b}")
                        for t8 in range(8):
                            t = b * 8 + t8
                            nc.tensor.transpose(
                                pst[:, t8 * P:(t8 + 1) * P],
                                pbf[:, t * P:(t + 1) * P],
                                ident16[:],
                            )
                        for hh in range(2):
                            dst = pT[:, b * 8 + hh * 4:b * 8 + (hh + 1) * 4, :]
                            src = pst[:, hh * 4 * P:(hh + 1) * 4 * P].rearrange(
                                "p (b q) -> p b q", b=4)
                            nc.vector.tensor_copy(dst, src)

                    # previous row's elementwise + stores go AFTER this row's
                    # pT drains in every engine queue, so the drains (which
                    # gate PE's ctx matmuls) never queue behind them
                    if prev_tail is not None:
                        prev_tail()
                    if i + 1 < NT:
                        emit_xblk(i + 1)

                    xblk = xblks[i]

                    # o_sb holds blocks [ctx, x+ctx, x-ctx, x*ctx]
                    o_sb = owork.tile([P, 4 * D], DT.float32, tag="o_sb",
                                      name=f"o_sb{i}")
                    tail = i >= NT - 3
                    for dc in range(2):
                        lo = dc * 512
                        psc = ps_c.tile([P, 512], DT.float32, tag="psc",
                                        name=f"psc{i}_{dc}")
                        for t in range(NT):
                            nc.tensor.matmul(
                                psc[:], pT[:, t, :], xnb[:, t, lo:lo + 512],
                                start=(t == 0), stop=(t == NT - 1),
                            )
                        if not tail:
                            nc.scalar.mul(o_sb[:, lo:lo + 512], psc[:], recip[:])
                        else:
                            # tail rows: DVE drain so the scale never queues
                            # behind the next row's ACT exps
                            nc.vector.tensor_scalar(
                                out=o_sb[:, lo:lo + 512], in0=psc[:],
                                scalar1=recip[:], scalar2=None,
                                op0=mybir.AluOpType.mult,
                            )

                    def out_tail():
                        for dc in range(2):
                            lo = dc * 512
                            xh = xblk[:, lo:lo + 512]
                            ch = o_sb[:, lo:lo + 512]
                            nc.gpsimd.tensor_add(
                                o_sb[:, D + lo:D + lo + 512], xh, ch)
                            if not tail:
                                nc.gpsimd.tensor_sub(
                                    o_sb[:, 2 * D + lo:2 * D + lo + 512], xh, ch)
                            else:
                                nc.vector.tensor_sub(
                                    o_sb[:, 2 * D + lo:2 * D + lo + 512], xh, ch)
                            nc.vector.tensor_mul(
                                o_sb[:, 3 * D + lo:3 * D + lo + 512], xh, ch)
                        # per-block stores: earlier starts, finer WAR release
                        for blk in range(4):
                            nc.sync.dma_start(
                                out[q_sl, (blk + 1) * D:(blk + 2) * D],
                                o_sb[:, blk * D:(blk + 1) * D],
                            )

                    if tail:
                        # A-work is exhausted here; run inline so the final
                        # stores drain the DMA backlog before the flush
                        out_tail()
                        return None
                    return out_tail

                # A two rows ahead of B
                LOOK = 2
                emit_xblk(0)
                for k in range(LOOK):
                    emit_scores_rest(k)
                pending_tail = None
                for i in range(NT):
                    if i + LOOK < NT:
                        emit_scores_rest(i + LOOK)
                    pending_tail = emit_rest(i, pending_tail,
                                             last=(i == NT - 1))

    nc.finalize()
    return nc


_NC_CACHE = None


def _get_nc():
    global _NC_CACHE
    if _NC_CACHE is None:
        _NC_CACHE = _build()
    return _NC_CACHE


def kernel(x, mask, _trace=False):
    x = np.asarray(x, dtype=np.float32)
    mask = np.asarray(mask, dtype=np.int32)
    assert x.shape == (NB, S, D), x.shape
    assert mask.shape == (NB, S), mask.shape

    nc = _get_nc()
    in_maps = [
        {"x": np.ascontiguousarray(x[b]), "mask": np.ascontiguousarray(mask[b])}
        for b in range(NB)
    ]
    res = run_bass_kernel_spmd(nc, in_maps, core_ids=list(range(NB)), trace=_trace)
    out = np.stack([r["out"] for r in res.results], axis=0)
    if _trace:
        return out, res
    return out
